# revision 1
# baseline (speedup 1.0000x reference)
"""Multi-head self-attention on 8 Trainium2 NeuronCores (raw Bass).

Problem: B=2, S=2048, D=1024, H=16 heads of depth 64 (fp32).
    q/k/v = x @ W.T + b ; per-head softmax(q k^T / 8) v ; dense out proj.

Sharding: DP-2 on batch x TP-4 on heads. Core (b, g) handles batch b and the
4 heads g*4..g*4+3 (a 256-wide column block of q/k/v). The dense layer is
row-split over the same block, so each core emits a partial [S, D] output;
the host sums the 4 partials per batch (dense bias rides on the g==0 cores).

Raw-bass implementation (the toolchain accepts at most ONE semaphore wait
per engine instruction, so Tile's multi-wait sync is unusable; explicit
wait_ge instructions + then_inc updates are used throughout):

  phase 1: stream x^T in [128, 512] slices; QT/KT/VT = W_T.T @ x^T in [c, s]
           layout (f32r matmuls, 6 psum accumulators); VT slices are
           PE-transposed into V_aug [s, c] with a ones column appended.
  attention (8 iters = head pair x 512-wide q block; 16 k tiles each):
           scores^T via row-packed pair matmuls (contraction=64, two heads
           share the 128-row PE array), exp on ScalarE (scale=1/8 folded in),
           PV accumulation with lhsT = V_aug; its ones column leaves the
           softmax denominators in psum row 64; normalize via reciprocal +
           a DRAM-bounce partition broadcast.
  dense:   per q block, partial^T = dwT.T @ OT (+ db) -> DMA out.
"""

import numpy as np
import sys

if "/opt/trn_rl_repo" not in sys.path:
    sys.path.insert(0, "/opt/trn_rl_repo")

import concourse.bass as bass
from concourse import mybir
from concourse.bass_utils import run_bass_kernel_spmd

F32 = mybir.dt.float32
F32R = mybir.dt.float32r
BF16 = mybir.dt.bfloat16
F8E4 = mybir.dt.float8e4
AFT = mybir.ActivationFunctionType

B, S, D = 2, 2048, 1024
H, DEPTH = 16, 64
TP = 4                     # head-parallel groups
C = D // TP                # 256 cols per core (4 heads)
CT = C // 128              # 2 partition tiles of the head block
KD = D // 128              # 8 contraction tiles for projections
ST_N = S // 128            # 16 s/k tiles
QB = 512                   # q block width
NQB = S // QB              # 4 q blocks
NPAIR = 2                  # head pairs per core
NITER = NQB * NPAIR        # 8 attention iterations (qp)
NE = NITER * ST_N          # 128 score-tile steps (e)
NX = 8                     # x slice ring depth
NP = 8                     # p tile ring depth (even: PV consumes aligned pairs)
SCALE = float(1.0 / np.sqrt(DEPTH))


EXP_SHIFT = -4.0           # exp(s/8 + EXP_SHIFT): keeps p under fp8e4 max


def build_nc():
    nc = bass.Bass()
    neg4 = nc.alloc_sbuf_tensor("neg4", [128, 1], F32)
    nc.gpsimd.memset(neg4.ap(), EXP_SHIFT)
    nc.all_engine_barrier()
    neg4 = neg4.ap()

    xT = nc.dram_tensor("xT", [D, S], BF16, kind="ExternalInput")
    wqT = nc.dram_tensor("wqT", [D, C], BF16, kind="ExternalInput")
    wkT = nc.dram_tensor("wkT", [D, C], BF16, kind="ExternalInput")
    wvT = nc.dram_tensor("wvT", [D, C], BF16, kind="ExternalInput")
    bq = nc.dram_tensor("bq", [C, 1], F32, kind="ExternalInput")
    bk = nc.dram_tensor("bk", [C, 1], F32, kind="ExternalInput")
    bv = nc.dram_tensor("bv", [C, 1], F32, kind="ExternalInput")
    dwT = nc.dram_tensor("dwT", [C, D], BF16, kind="ExternalInput")
    db = nc.dram_tensor("db", [D, 1], F32, kind="ExternalInput")
    identity = nc.dram_tensor("identity", [128, 128], F32R, kind="ExternalInput")
    outT = nc.dram_tensor("outT", [D, S], F32, kind="ExternalOutput")

    # ---- SBUF ----
    wq_sb = nc.alloc_sbuf_tensor("wq_sb", [128, KD, C], BF16).ap()
    wk_sb = nc.alloc_sbuf_tensor("wk_sb", [128, KD, C], BF16).ap()
    wv_sb = nc.alloc_sbuf_tensor("wv_sb", [128, KD, C], BF16).ap()
    bq_sb = nc.alloc_sbuf_tensor("bq_sb", [128, CT, 1], F32).ap()
    bk_sb = nc.alloc_sbuf_tensor("bk_sb", [128, CT, 1], F32).ap()
    bv_sb = nc.alloc_sbuf_tensor("bv_sb", [128, CT, 1], F32).ap()
    dw_sb = nc.alloc_sbuf_tensor("dw_sb", [128, CT, D], BF16).ap()
    db_sb = nc.alloc_sbuf_tensor("db_sb", [128, KD, 1], F32).ap()
    ident = nc.alloc_sbuf_tensor("ident", [128, 128], F32R).ap()
    x_ring = nc.alloc_sbuf_tensor("x_ring", [128, NX, QB], BF16).ap()
    qt_sb = nc.alloc_sbuf_tensor("qt_sb", [128, CT, S], BF16).ap()
    kt_sb = nc.alloc_sbuf_tensor("kt_sb", [128, CT, S], BF16).ap()
    vt_roll = nc.alloc_sbuf_tensor("vt_roll", [128, 2, QB], F32R).ap()
    # bf16 V/P (fp8e4 measured 4e-2 rel err vs the 2e-2 gate — p and V
    # rounding each contribute ~2.3e-2, so DoubleRow PV is out of budget)
    vaug = nc.alloc_sbuf_tensor("vaug", [128, TP, ST_N, 80], BF16).ap()
    p_ring = nc.alloc_sbuf_tensor("p_ring", [128, NP, 2, QB], BF16).ap()
    inv_bc = nc.alloc_sbuf_tensor("inv_bc", [64, 2, QB], F32).ap()
    sel = nc.alloc_sbuf_tensor("sel", [128, DEPTH], F32R).ap()
    tmp_sb = nc.alloc_sbuf_tensor("tmp_sb", [64, 2, QB], BF16).ap()
    ot = nc.alloc_sbuf_tensor("ot", [128, 2, CT, QB], BF16).ap()
    # f32r so the PV spill is a legal f32r-matmul input (denominator bc)
    ounorm = nc.alloc_sbuf_tensor("ounorm", [128, 2, 2, QB], F32R).ap()
    stage = nc.alloc_sbuf_tensor("stage", [128, 2, KD, QB], F32).ap()

    # ---- PSUM: one [128, 8, 512] tensor, banks managed manually ----
    # phase 1: banks 0-5 = projection accumulators (w*2+ct), 6-7 = transposes
    # attention: banks 0-3 = score tiles (slot b*2+head), 4-5 = PV accum A/B,
    #            6-7 = dense ring
    psum = nc.alloc_psum_tensor("ps", [128, 8, QB], F32).ap()

    # ---- semaphores ----
    # DMA completions across HW queues are out-of-order, so every DMA
    # stream with >1 outstanding transfer gets per-slot semaphores: each
    # wait value then corresponds to a deterministic set of completions.
    s = {n: nc.alloc_semaphore(n) for n in (
        "s_wq", "s_wk", "s_wv", "s_misc", "s_xcons", "s_cpv", "s_cpqk", "s_tr",
        "s_trcp",
        "s_st", "s_exp", "s_pcons", "s_acs", "s_nrm", "s_inv", "s_bc", "s_dn",
        "s_stg", "s_init")}
    s_x = [nc.alloc_semaphore(f"s_x{j}") for j in range(NX)]
    s_ot = [nc.alloc_semaphore(f"s_ot{p}") for p in range(2)]
    s_out = [nc.alloc_semaphore(f"s_out{p}") for p in range(2)]

    projs = ((wq_sb, bq_sb, 0), (wk_sb, bk_sb, 1), (wv_sb, bv_sb, 2))

    with nc.Block() as block:

        # ---------------- SP: all HWDGE DMA traffic ----------------
        @block.sync
        def _(sync):
            # inputs: wq + first x slices first so PE starts ASAP
            def emit_x(i):
                n, k = divmod(i, KD)
                if i >= NX:
                    sync.wait_ge(s["s_xcons"], i - (NX - 1))
                sync.dma_start(
                    out=x_ring[:, i % NX, :],
                    in_=xT[k * 128:(k + 1) * 128, n * QB:(n + 1) * QB],
                ).then_inc(s_x[i % NX], 16)

            sync.dma_start(
                out=wq_sb, in_=wqT.ap().rearrange("(k p) c -> p k c", p=128)
            ).then_inc(s["s_wq"], 16)
            for i in range(NX):
                emit_x(i)
            sync.dma_start(
                out=wk_sb, in_=wkT.ap().rearrange("(k p) c -> p k c", p=128)
            ).then_inc(s["s_wk"], 16)
            sync.dma_start(
                out=wv_sb, in_=wvT.ap().rearrange("(k p) c -> p k c", p=128)
            ).then_inc(s["s_wv"], 16)
            with nc.allow_non_contiguous_dma(reason="tiny bias vectors"):
                sync.dma_start(
                    out=bq_sb, in_=bq.ap().rearrange("(ct p) o -> p ct o", p=128)
                ).then_inc(s["s_misc"], 16)
                sync.dma_start(
                    out=bk_sb, in_=bk.ap().rearrange("(ct p) o -> p ct o", p=128)
                ).then_inc(s["s_misc"], 16)
                sync.dma_start(
                    out=bv_sb, in_=bv.ap().rearrange("(ct p) o -> p ct o", p=128)
                ).then_inc(s["s_misc"], 16)
            sync.dma_start(
                out=dw_sb, in_=dwT.ap().rearrange("(ct p) e -> p ct e", p=128)
            ).then_inc(s["s_misc"], 16)
            with nc.allow_non_contiguous_dma(reason="tiny bias vector"):
                sync.dma_start(
                    out=db_sb, in_=db.ap().rearrange("(m p) o -> p m o", p=128)
                ).then_inc(s["s_misc"], 16)
            sync.dma_start(out=ident, in_=identity.ap()).then_inc(s["s_misc"], 16)
            for i in range(NX, NQB * KD):
                emit_x(i)

            # attention-side: head B partition shift tmp -> ot rows 64..127
            for qb in range(NQB):
                for pair in range(NPAIR):
                    qp = qb * NPAIR + pair
                    if qb >= 2 and pair == 0:
                        sync.wait_ge(s["s_dn"], 8 * (qb - 1))  # ot slot WAR
                    sync.wait_ge(s["s_nrm"], 2 * qp + 2)
                    sync.dma_start(
                        out=ot[64:128, qb % 2, pair, :], in_=tmp_sb[:, qp % 2, :]
                    ).then_inc(s_ot[qp % 2], 16)
                # output DMAs
                for m8 in range(KD):
                    d = qb * KD + m8
                    sync.wait_ge(s["s_stg"], d + 1)
                    sync.dma_start(
                        out=outT[m8 * 128:(m8 + 1) * 128, qb * QB:(qb + 1) * QB],
                        in_=stage[:, qb % 2, m8, :],
                    ).then_inc(s_out[qb % 2], 16)

        # ---------------- PE: matmuls + transposes ----------------
        @block.tensor
        def _(tensor):
            # v-adds (DVE, s_cpv) free banks 4-5; q/k adds (ScalarE,
            # s_cpqk) free banks 0-3; PE touches banks in free order
            mm_order = ((2, 0), (2, 1), (0, 0), (0, 1), (1, 0), (1, 1))
            bank_free = {(2, 0): ("s_cpv", 2, 1), (2, 1): ("s_cpv", 2, 2),
                         (0, 0): ("s_cpqk", 4, 1), (0, 1): ("s_cpqk", 4, 2),
                         (1, 0): ("s_cpqk", 4, 3), (1, 1): ("s_cpqk", 4, 4)}
            w_sem = {0: "s_wq", 1: "s_wk", 2: "s_wv"}
            w_seen = set()
            for n in range(NQB):
                for k in range(KD):
                    i = n * KD + k
                    tensor.wait_ge(s_x[i % NX], 16 * (i // NX + 1))
                    last = None
                    for w, ct in mm_order:
                        if w not in w_seen:
                            w_seen.add(w)
                            tensor.wait_ge(s[w_sem[w]], 16)
                        if n >= 1 and k == 0:
                            sem, per, off = bank_free[w, ct]
                            tensor.wait_ge(s[sem], per * (n - 1) + off)
                        last = nc.tensor.matmul(
                            psum[:, w * 2 + ct, :],
                            (projs[w][0])[:, k, ct * 128:(ct + 1) * 128],
                            x_ring[:, i % NX, :],
                            start=(k == 0), stop=(k == KD - 1),
                        )
                    last.then_inc(s["s_xcons"], 1)
                # V transposes for this n: t = n*8 + ct*4 + j
                if n == 0:
                    tensor.wait_ge(s["s_misc"], 96)  # identity loaded
                for ct in range(CT):
                    tensor.wait_ge(s["s_cpv"], 2 * n + 1 + ct)  # vt_roll ready
                    for j in range(QB // 128):
                        t = n * (2 * (QB // 128)) + ct * (QB // 128) + j
                        if t >= 2:
                            tensor.wait_ge(s["s_trcp"], t - 1)
                        nc.tensor.transpose(
                            psum[:, 6 + t % 2, 0:128].bitcast(F32R),
                            vt_roll[:, (2 * n + ct) % 2, j * 128:(j + 1) * 128],
                            ident,
                        ).then_inc(s["s_tr"], 1)

            # phase 1 psum fully consumed before attention reuses the banks
            tensor.wait_ge(s["s_cpqk"], 4 * NQB)
            tensor.wait_ge(s["s_cpv"], 2 * NQB)
            tensor.wait_ge(s["s_trcp"], 2 * (QB // 128) * NQB)

            # attention — software-pipelined: QK(e+1) issues before PV(e),
            # so PE never stalls on ScalarE's exp; PV accumulators are
            # spilled to SBUF by DVE (s_acs) so normalization (reciprocal +
            # DRAM-bounce broadcast) runs entirely off PE's critical path.
            def emit_qk(e):
                qp, m = divmod(e, ST_N)
                qb, pair = divmod(qp, NPAIR)
                b = e % 2
                msl = slice(m * 128, (m + 1) * 128)
                qsl = slice(qb * QB, (qb + 1) * QB)
                if e >= 2:
                    tensor.wait_ge(s["s_exp"], e - 1)
                nc.tensor.matmul(
                    psum[:, b * 2 + 0, :],
                    kt_sb[0:64, pair, msl],
                    qt_sb[0:64, pair, qsl],
                    start=True, stop=True, tile_position=(0, 0),
                )
                nc.tensor.matmul(
                    psum[:, b * 2 + 1, :],
                    kt_sb[64:128, pair, msl],
                    qt_sb[64:128, pair, qsl],
                    start=True, stop=True, tile_position=(64, 0),
                ).then_inc(s["s_st"], 1)

            def emit_pv_pair(e0):
                # one DoubleRow matmul per head covers e-steps (e0, e0+1):
                # two k-tiles packed, fp8 at 0.5 cycles/col
                qp, m0 = divmod(e0, ST_N)
                pair = qp % NPAIR
                sl = e0 % NP
                tensor.wait_ge(s["s_exp"], e0 + 2)
                if m0 == 0 and qp >= 1:
                    tensor.wait_ge(s["s_acs"], 2 * qp)  # acc spilled to SBUF
                # [V|ones] per head -> rows 0..64, denominators in row 64
                last = None
                for j in range(2):
                    m = m0 + j
                    for h in range(2):
                        last = nc.tensor.matmul(
                            psum[0:65, 4 + h, :],
                            vaug[:, 2 * pair + h, m, 0:DEPTH + 1],
                            p_ring[:, (sl + j) % NP, h, :],
                            start=(m == 0), stop=(m == ST_N - 1),
                        )
                last.then_inc(s["s_pcons"], 2)

            def emit_bc(qp):
                # broadcast the denominator (spilled row 64) to rows 0..63
                # via contraction-64 matmuls against a one-hot selector;
                # garbage lanes hit zero selector rows (ounorm background is
                # memset to 1.0 so no NaN*0); DVE then reciprocals the
                # broadcast in place on 64 partitions
                qb = qp // NPAIR
                if qp == 0:
                    tensor.wait_ge(s["s_init"], 1)
                if qp % NPAIR == 0:
                    if qb >= 1:
                        tensor.wait_ge(s["s_stg"], 8 * qb)  # stage(qb-1) read 6/7
                else:
                    tensor.wait_ge(s["s_nrm"], 4 * qb + 2)  # pair0 muls read 6/7
                tensor.wait_ge(s["s_acs"], 2 * qp + 2)  # spills retired
                for h in range(2):
                    mm = nc.tensor.matmul(
                        psum[0:64, 6 + h, :],
                        sel[64:128, :],
                        ounorm[64:128, qp % 2, h, :],
                        start=True, stop=True, tile_position=(64, 0),
                    )
                mm.then_inc(s["s_bc"], 2)

            def emit_dense_step(qb, m8):
                if m8 == 0:
                    tensor.wait_ge(s["s_nrm"], 4 * qb + 4)
                    tensor.wait_ge(s_ot[0], 16 * (qb + 1))
                    tensor.wait_ge(s_ot[1], 16 * (qb + 1))
                d = qb * KD + m8
                if d >= 2:
                    tensor.wait_ge(s["s_stg"], d - 1)
                nc.tensor.matmul(
                    psum[:, 6 + d % 2, :],
                    dw_sb[:, 0, m8 * 128:(m8 + 1) * 128],
                    ot[:, qb % 2, 0, :],
                    start=True, stop=False,
                )
                nc.tensor.matmul(
                    psum[:, 6 + d % 2, :],
                    dw_sb[:, 1, m8 * 128:(m8 + 1) * 128],
                    ot[:, qb % 2, 1, :],
                    start=False, stop=True,
                ).then_inc(s["s_dn"], 1)

            # bc(qp) lands 2 e-steps after qp's last PV (normalization chain
            # latency hidden); dense(qb) one pair later, spread one m8-step
            # per e so ScalarE's exp stream never starves. The spread must
            # end before the NEXT qp's first PV slot (e = (2qb+3)*16) — its
            # stage ops sit before that qp's spills in DVE program order
            # (validated by the sync-graph sim in the dev notes).
            bc_at = {ST_N * (qp + 1) + 2: qp for qp in range(NITER - 1)}
            dense_sched = {}
            for qb in range(NQB - 1):
                for m8 in range(KD):
                    dense_sched[(2 * qb + 2) * ST_N + 7 + m8] = (qb, m8)
            for e in range(NE):
                emit_qk(e)
                if e >= 2 and e % 2 == 0:
                    emit_pv_pair(e - 2)
                if e in bc_at:
                    emit_bc(bc_at[e])
                if e in dense_sched:
                    emit_dense_step(*dense_sched[e])
            emit_pv_pair(NE - 2)
            emit_bc(NITER - 1)
            for m8 in range(KD):
                emit_dense_step(NQB - 1, m8)

        # ---------------- ACT: q/k bias adds (phase 1) + exp ----------------
        @block.scalar
        def _(scalar):
            scalar.wait_ge(s["s_misc"], 96)
            for n in range(NQB):
                nsl = slice(n * QB, (n + 1) * QB)
                scalar.wait_ge(s["s_xcons"], KD * (n + 1))
                for w_sb, b_sb, w in projs[:2]:
                    dst = qt_sb if w == 0 else kt_sb
                    for ct in range(CT):
                        nc.scalar.activation(
                            out=dst[:, ct, nsl],
                            in_=psum[:, w * 2 + ct, :],
                            func=AFT.Identity, bias=b_sb[:, ct, :], scale=1.0,
                        ).then_inc(s["s_cpqk"], 1)
            for e in range(NE):
                b = e % 2
                scalar.wait_ge(s["s_st"], e + 1)
                if e >= NP:
                    # slot e%NP freed by the PV pair covering (e-NP, e-NP+1)
                    scalar.wait_ge(s["s_pcons"], e - (NP - 2))
                # -4 keeps exp() under fp8e4's 448 max (score max ~8.7);
                # softmax shift-invariance cancels it in the normalization
                nc.scalar.activation(
                    out=p_ring[:, e % NP, :, :],
                    in_=psum[:, b * 2:b * 2 + 2, :],
                    func=AFT.Exp, scale=SCALE, bias=neg4,
                ).then_inc(s["s_exp"], 1)

        # ---------------- DVE: bias adds, copies, normalize, stage ----------
        @block.vector
        def _(vector):
            for hh in range(TP):
                for st_i in range(ST_N):
                    nc.vector.memset(vaug[:, hh, st_i, DEPTH:DEPTH + 1], 1.0)
            # recip background must be NaN-free (bc selector zeros them out)
            nc.vector.memset(ounorm.bitcast(F32), 1.0)
            nc.vector.memset(sel.bitcast(F32), 0.0)
            nc.vector.memset(sel[64:65, :].bitcast(F32), 1.0).then_inc(
                s["s_init"], 1)
            vector.wait_ge(s["s_misc"], 96)
            # phase 1 (v adds first: PE's transposes + next group wait on them)
            for n in range(NQB):
                nsl = slice(n * QB, (n + 1) * QB)
                vector.wait_ge(s["s_xcons"], KD * (n + 1))
                for ct in range(CT):
                    # vt_roll WAR: transposes of the slot two groups back
                    g = 2 * n + ct
                    if g >= 2:
                        prev = g - 2  # = 2n'+ct'
                        tprev = (prev // 2) * (2 * (QB // 128)) \
                            + (prev % 2) * (QB // 128) + (QB // 128 - 1)
                        vector.wait_ge(s["s_tr"], tprev + 1)
                    nc.vector.tensor_scalar_add(
                        out=vt_roll[:, g % 2, :],
                        in0=psum[:, 4 + ct, :],
                        scalar1=bv_sb[:, ct, :],
                    ).then_inc(s["s_cpv"], 1)
                # V_aug assembly from transposed tiles
                for ct in range(CT):
                    for j in range(QB // 128):
                        t = n * (2 * (QB // 128)) + ct * (QB // 128) + j
                        st_i = n * (QB // 128) + j
                        vector.wait_ge(s["s_tr"], t + 1)
                        nc.vector.tensor_copy(
                            out=vaug[:, 2 * ct, st_i, 0:DEPTH],
                            in_=psum[:, 6 + t % 2, 0:DEPTH],
                        )
                        nc.vector.tensor_copy(
                            out=vaug[:, 2 * ct + 1, st_i, 0:DEPTH],
                            in_=psum[:, 6 + t % 2, DEPTH:128],
                        ).then_inc(s["s_trcp"], 1)

            # attention: normalization + dense staging
            for qb in range(NQB):
                for pair in range(NPAIR):
                    qp = qb * NPAIR + pair
                    vector.wait_ge(s["s_pcons"], ST_N * (qp + 1))
                    if qp >= 2:
                        # ounorm slot WAR: bc (PE) of qp-2 read it last
                        vector.wait_ge(s["s_bc"], 2 * (qp - 2) + 2)
                    nc.vector.tensor_copy(
                        out=ounorm[0:65, qp % 2, 0, :],
                        in_=psum[0:65, 4, :],
                    ).then_inc(s["s_acs"], 1)
                    nc.vector.tensor_copy(
                        out=ounorm[0:65, qp % 2, 1, :],
                        in_=psum[0:65, 5, :],
                    ).then_inc(s["s_acs"], 1)
                    vector.wait_ge(s["s_bc"], 2 * qp + 2)  # denom bc landed
                    for h in range(2):
                        nc.vector.reciprocal_approx_fast(
                            out=inv_bc[:, h, :],
                            in_=psum[0:64, 6 + h, :],
                        ).then_inc(s["s_inv"], 1)
                    # self-wait: DVE's deep pipe may start the next read
                    # before the recip writes retire; sem incs fire at retire
                    vector.wait_ge(s["s_inv"], 2 * (qp + 1))
                    for h in range(2):
                        if pair == 0 and h == 0 and qb >= 2:
                            vector.wait_ge(s["s_dn"], 8 * (qb - 1))  # ot WAR
                        if h == 0:
                            nc.vector.tensor_mul(
                                out=ot[0:64, qb % 2, pair, :],
                                in0=ounorm[0:64, qp % 2, 0, :].bitcast(F32),
                                in1=inv_bc[:, 0, :],
                            ).then_inc(s["s_nrm"], 1)
                        else:
                            if qp >= 2:
                                # tmp slot WAR: shift DMA of qp-2 done
                                vector.wait_ge(s_ot[qp % 2], 16 * (qp // 2))
                            nc.vector.tensor_mul(
                                out=tmp_sb[:, qp % 2, :],
                                in0=ounorm[0:64, qp % 2, 1, :].bitcast(F32),
                                in1=inv_bc[:, 1, :],
                            ).then_inc(s["s_nrm"], 1)
                for m8 in range(KD):
                    d = qb * KD + m8
                    vector.wait_ge(s["s_dn"], d + 1)
                    if qb >= 2 and m8 == 0:
                        # stage slot WAR: all of q block qb-2's output DMAs
                        vector.wait_ge(s_out[qb % 2], 16 * KD * (qb // 2))
                    nc.vector.tensor_scalar_add(
                        out=stage[:, qb % 2, m8, :],
                        in0=psum[:, 6 + d % 2, :],
                        scalar1=db_sb[:, m8, :],
                    ).then_inc(s["s_stg"], 1)

    nc.finalize()
    # custom-DVE ops (reciprocal_approx_fast) lower to InstISA subclasses
    # whose .instr bytes raw Bass doesn't generate
    mybir.codegen_inst_isa_subclasses(nc)
    return nc


_NC_CACHE = []


def get_nc():
    if not _NC_CACHE:
        _NC_CACHE.append(build_nc())
    return _NC_CACHE[0]


def make_in_maps(x, wq_w, wq_b, wk_w, wk_b, wv_w, wv_b, dense_w, dense_b):
    import ml_dtypes
    bf16 = ml_dtypes.bfloat16
    in_maps = []
    for core in range(8):
        b, g = divmod(core, TP)
        blk = slice(g * C, (g + 1) * C)
        db_g = dense_b if g == 0 else np.zeros_like(dense_b)
        in_maps.append({
            "xT": np.ascontiguousarray(x[b].T.astype(bf16)),
            "wqT": np.ascontiguousarray(wq_w[blk, :].T.astype(bf16)),
            "wkT": np.ascontiguousarray(wk_w[blk, :].T.astype(bf16)),
            "wvT": np.ascontiguousarray(wv_w[blk, :].T.astype(bf16)),
            "bq": np.ascontiguousarray(wq_b[blk].reshape(C, 1)),
            "bk": np.ascontiguousarray(wk_b[blk].reshape(C, 1)),
            "bv": np.ascontiguousarray(wv_b[blk].reshape(C, 1)),
            "dwT": np.ascontiguousarray(dense_w[:, blk].T.astype(bf16)),
            "db": np.ascontiguousarray(db_g.reshape(D, 1)),
            "identity": np.eye(128, dtype=np.float32),
        })
    return in_maps


def gather_out(results):
    out = np.zeros((B, S, D), dtype=np.float32)
    for core in range(8):
        b = core // TP
        out[b] += results[core]["outT"].T
    return out


def kernel(x, wq_w, wq_b, wk_w, wk_b, wv_w, wv_b, dense_w, dense_b, **run_kwargs):
    args = [np.asarray(a, dtype=np.float32) for a in (
        x, wq_w, wq_b, wk_w, wk_b, wv_w, wv_b, dense_w, dense_b)]
    nc = get_nc()
    in_maps = make_in_maps(*args)
    res = run_bass_kernel_spmd(nc, in_maps, list(range(8)), **run_kwargs)
    out = gather_out(res.results)
    kernel.last_results = res
    return out

